# revision 32
# baseline (speedup 1.0000x reference)
"""Trainium2 Bass kernel for the sparse video-attention module.

Model (reference):
    k = conv3x3(x[:, 0], w_k)                     # key from first frame only
    q = conv3x3(x, w_q); v = conv3x3(x, w_v)      # per-frame
    dots[b,t,h,w] = sum_c q[b,t,c,h,w] * k[b,c,h,w]
    attn = softmax_T(dots)
    pooled = sum_t attn[...,t] * v[...,t]         # (B, DH, H, W)
    out = conv3x3(pooled, w_out) + b_out          # identical for every t

Sharding: 8 cores = (batch b in 0..3) x (row half in 0..1). Each core owns 32
output rows of one batch element; all coupling (softmax over T, convs) is
local given a 2-row input halo, so there is no inter-core communication.
The host pre-pads/slices inputs per core and re-assembles + broadcasts the
output over T at the end. The lower-half cores see their slab VERTICALLY
FLIPPED (x rows, conv-kernel kh taps, and the output rows are flipped on the
host): that puts the out-of-image zero-pad row at pooled row 0 on EVERY
core, so the single SPMD program computes only the 33 live rows and memsets
row 0 -- no per-core divergence and no wasted conv row.

Per-core layout: pixels of compute rows 1..33 (32 own rows + the lower halo)
are flattened row-major; convs are matmuls with fp16 operands (full PE rate,
halves DMA vs fp32) with C on the contraction dim (2 chunks of 128) and 9
spatial taps accumulated in PSUM; spatial shifts are plain access-pattern
offsets into a zero-padded [128, 36, 66] SBUF image. Conv loops run
row-block-outer so each PSUM bank completes after its 18 taps and downstream
engines drain it while the PE continues. A short warm-up matmul train covers
the p-state ramp while the first (chunked, interleaved) DMAs land.

The channel reduction for dots runs on the idle GPSIMD engine
(partition_all_reduce) instead of the PE; a tiny DMA drops each frame's row
onto partition t of a [8, NPIX] tile, so the softmax over T is a GPSIMD
cross-partition max/sum plus DVE/ACT elementwise work in that transposed
layout -- no PE transposes, no ones-matmuls. Normalization is deferred: the
exp rows broadcast to all 128 partitions via gpsimd partition_broadcast (no
DRAM bounce) right after the ACT exp, the v-pass accumulates exp_t * v_t
straight from the conv PSUM banks on the Vector engine (hidden under the
next frame's matmuls), and only the last frame's accumulation multiplies by
the broadcast 1/sum while writing the fp16 pooled copy the out-conv reads.
This keeps the softmax chain off the PE critical path at the phase boundary.
"""

import sys

import numpy as np

for _p in ("/opt/trn_rl_repo", "/root/.axon_site/_ro/trn_rl_repo"):
    if _p not in sys.path:
        sys.path.insert(0, _p)

B, T, C, H, W = 4, 8, 256, 64, 64
DH = 128
NCORES = 8
CR = 34            # compute rows per core (32 out rows + 1 halo row each side)
XR = 36            # x rows per core (compute rows + 1 conv-halo row each side)
WP = W + 2         # zero-padded width
NPIX = CR * W      # 2176 pixel slots per core (row 0's 64 are dead)
# Compute row blocks: pooled rows 1..33. Row 0 (the out-of-image conv-pad
# row) is never computed -- the host flips the lower-half cores' data
# vertically so the dead row is row 0 on EVERY core, and the kernel just
# zeroes it. Saves a conv row on all 17 frame-convs.
RB = [(1, 8), (9, 8), (17, 8), (25, 8), (33, 1)]
# over the 32 output rows; the tiny last block shortens the end-of-kernel
# ACT+DMA drain tail after the final matmul
OUT_RB = [(0, 8), (8, 8), (16, 8), (24, 6), (30, 2)]

RUN_KWARGS: dict = {}   # extra kwargs for run_bass_kernel_spmd (test hook)
LAST_RESULT = None      # last BassKernelResults (test hook)

_cache: dict = {}


def _build_nc():
    from contextlib import ExitStack

    import concourse.mybir as mybir
    import concourse.tile as tile
    from concourse import bacc, bass_isa

    f32 = mybir.dt.float32
    f16 = mybir.dt.float16
    AF = mybir.ActivationFunctionType
    RO = bass_isa.ReduceOp

    nc = bacc.Bacc("TRN2", target_bir_lowering=False)
    kv_dsem = nc.alloc_semaphore("kv_dsem")

    xs_d = nc.declare_dram_parameter("xs", [T, 2, 128, XR * WP], f16, isOutput=False)
    wq_d = nc.declare_dram_parameter("wq", [128, 2, 9, 128], f16, isOutput=False)
    wk_d = nc.declare_dram_parameter("wk", [128, 2, 9, 128], f16, isOutput=False)
    wv_d = nc.declare_dram_parameter("wv", [128, 2, 9, 128], f16, isOutput=False)
    wo_d = nc.declare_dram_parameter("wo", [128, 9, 256], f16, isOutput=False)
    bo_d = nc.declare_dram_parameter("bo", [128, 2], f32, isOutput=False)
    out_d = nc.declare_dram_parameter("out", [2, 128, 32 * W], f32, isOutput=True)

    with tile.TileContext(nc) as tc, ExitStack() as ctx:
        singles = ctx.enter_context(tc.tile_pool(name="singles", bufs=1))
        xpool = ctx.enter_context(tc.tile_pool(name="xpool", bufs=7))
        sb = ctx.enter_context(tc.tile_pool(name="sb", bufs=1))
        qkpool = ctx.enter_context(tc.tile_pool(name="qkpool", bufs=2))
        reppool = ctx.enter_context(tc.tile_pool(name="reppool", bufs=2))
        scrpool = ctx.enter_context(tc.tile_pool(name="scrpool", bufs=2))
        abpool = ctx.enter_context(tc.tile_pool(name="abpool", bufs=8))
        psc = ctx.enter_context(tc.tile_pool(name="psc", bufs=8, space="PSUM"))

        nc.gpsimd.sem_clear(kv_dsem)
        # PE warm-up: short matmuls on a zeroed tile ride out the p-state
        # ramp while the first DMAs are still in flight.
        warm = singles.tile([128, 128], f16, tag="warm")
        nc.vector.memset(warm, 0.0)
        wps = psc.tile([128, 512], f32, tag="cv", name="warmps")
        for i in range(28):
            nc.tensor.matmul(wps[:, :128], warm, warm, start=True, stop=True)

        def load_x(t, chunks=1, interleave=None):
            """Load frame t as two [128, XR*WP] fp16 tiles (one per C-group).

            chunks>1 splits each group's DMA into row chunks; `interleave`
            is a list of (idx, fn) callbacks issued between chunk DMAs so the
            first-needed weight DMAs land between the first x chunks.
            """
            tiles = [
                xpool.tile([128, XR * WP], f16, tag="xt", name=f"xt{t}_{g}")
                for g in range(2)
            ]
            rows = XR // chunks
            n = 0
            for c in range(chunks):
                sl = slice(c * rows * WP, (c + 1) * rows * WP)
                for g in range(2):
                    if interleave:
                        for idx, fn in interleave:
                            if idx == n:
                                fn()
                    nc.sync.dma_start(out=tiles[g][:, sl], in_=xs_d[t, g, :, sl])
                    n += 1
            return [xt.rearrange("p (r c) -> p r c", c=WP) for xt in tiles]

        wk_sb = singles.tile([128, 2, 9, 128], f16, tag="wk")
        wq_sb = singles.tile([128, 2, 9, 128], f16, tag="wq")
        wv_sb = singles.tile([128, 2, 9, 128], f16, tag="wv")
        wo_sb = singles.tile([128, 9, 256], f16, tag="wo")
        bo_sb = singles.tile([128, 2], f32, tag="bo")

        # First x chunks and the k-conv weights interleave so the PE's first
        # real matmul (k conv, row-block 0, group 0) starts ~3us in.
        x0 = load_x(
            0,
            chunks=3,
            interleave=[
                (0, lambda: nc.sync.dma_start(out=wk_sb[:, 0, :3], in_=wk_d[:, 0, :3])),
                (1, lambda: nc.sync.dma_start(out=wk_sb[:, 0, 3:], in_=wk_d[:, 0, 3:])),
                (2, lambda: nc.sync.dma_start(out=wk_sb[:, 1], in_=wk_d[:, 1])),
            ],
        )
        nc.sync.dma_start(out=wq_sb, in_=wq_d[:])
        nc.sync.dma_start(out=wv_sb, in_=wv_d[:])
        nc.sync.dma_start(out=wo_sb, in_=wo_d[:])
        nc.sync.dma_start(out=bo_sb, in_=bo_d[:])

        def conv3x3(psums, xt3, w_sb, done=None):
            # Row-block-outer: psums[r] finishes after its 18 taps so
            # downstream (DVE/ACT) drains bank r while the PE runs r+1.
            for r, (R0, nr) in enumerate(RB):
                for g in range(2):
                    for j in range(9):
                        ky, kx = divmod(j, 3)
                        nc.tensor.matmul(
                            psums[r][:, : nr * W],
                            w_sb[:, g, j, :],
                            xt3[g][:, R0 + ky : R0 + ky + nr, kx : kx + W],
                            start=(g == 0 and j == 0),
                            stop=(g == 1 and j == 8),
                        )
                if done is not None:
                    done(r, R0, nr)

        k_sb = sb.tile([128, NPIX], f32, tag="k")
        kvidx = singles.tile([128, 1], mybir.dt.int32, tag="kvidx")
        nc.vector.memset(kvidx, 30 * W)
        kvscr = singles.tile([128, 128], f32, tag="kvscr")
        dots8 = sb.tile([8, NPIX], f32, tag="dots8")
        red8 = sb.tile([8, NPIX], f32, tag="red8")
        pooled = sb.tile([128, CR, WP], f32, tag="pooled")
        pooled_h = sb.tile([128, CR, WP], f16, tag="pooled_h")
        out_sb = sb.tile([128, 2, 32 * W], f32, tag="out")

        # The final out block leaves via a SWDGE kv_writeback: descriptors
        # are generated here (before out_sb is ever written, so this
        # schedules early) and triggered after the last bias-add -- the
        # trigger path skips the HWDGE + DGE-delay latency on the drain.
        nc.gpsimd.kv_writeback(
            out_d[1:2].rearrange("b p (a n) -> b p a n", a=1),
            out_sb[:, 1:2, 30 * W : 32 * W].rearrange("p a (b n) -> p a b n", b=1),
            kvidx,
            prepare_only=True,
            sem=kv_dsem,
        )

        # ---- phase 1: k = conv(x[0], w_k) ----
        kps = [
            psc.tile([128, 512], f32, tag="cv", name=f"kps{r}")
            for r in range(len(RB))
        ]

        def k_done(r, R0, nr):
            nc.scalar.activation(
                k_sb[:, R0 * W : (R0 + nr) * W], kps[r][:, : nr * W], AF.Copy
            )

        conv3x3(kps, x0, wk_sb, done=k_done)

        # ---- phase 2: per frame q conv + dots ----
        for t in range(T):
            xt = x0 if t == 0 else load_x(t)
            qps = [
                psc.tile([128, 512], f32, tag="cv", name=f"qps{t}_{r}")
                for r in range(len(RB))
            ]
            qk = qkpool.tile([128, NPIX], f32, tag="qk", name=f"qk{t}")

            def q_done(r, R0, nr):
                cols = slice(R0 * W, (R0 + nr) * W)
                nc.vector.tensor_mul(qk[:, cols], qps[r][:, : nr * W], k_sb[:, cols])

            # dots[t, pix] = sum_c qk[c, pix] on the idle GPSIMD engine,
            # per row-block so the reduce starts before the frame finishes;
            # a tiny contiguous DMA then drops row 0 onto partition t.
            rep = reppool.tile([128, NPIX], f32, tag="rep", name=f"rep{t}")

            def qd_done(r, R0, nr):
                cols = slice(R0 * W, (R0 + nr) * W)
                q_done(r, R0, nr)
                nc.gpsimd.partition_all_reduce(
                    rep[:, cols], qk[:, cols], channels=128, reduce_op=RO.add
                )

            conv3x3(qps, xt, wq_sb, done=qd_done)
            nc.sync.dma_start(out=dots8[t : t + 1, W:], in_=rep[0:1, W:])

        def vconv(t, xt):
            vps = [
                psc.tile([128, 512], f32, tag="cv", name=f"vps{t}_{r}")
                for r in range(len(RB))
            ]
            conv3x3(vps, xt, wv_sb)
            return vps

        # v conv for frame 0 keeps the PE busy through the softmax below;
        # reuses the still-resident phase-1 x0 tiles (no reload).
        vps0 = vconv(0, x0)

        # ---- softmax over t = the 8 partitions of dots8, normalization
        # deferred: phase 3 accumulates exp(d-max)*v and the final frame's
        # write folds in rsm = mask/sum, so the exp rows broadcast as soon as
        # the ACT exp lands -- the sum/recip runs off the critical path.
        nc.gpsimd.partition_all_reduce(
            red8[:, W:], dots8[:, W:], channels=8, reduce_op=RO.max
        )
        nc.vector.tensor_sub(dots8[:, W:], dots8[:, W:], red8[:, W:])
        nc.scalar.activation(dots8[:, W:], dots8[:, W:], AF.Exp)
        scr0 = scrpool.tile([1, NPIX], f32, tag="scr", name="scr0")
        nc.sync.dma_start(out=scr0[:, W:], in_=dots8[0:1, W:])
        ab0 = [
            abpool.tile([128, 512], f32, tag="ab", name=f"ab0_{r}")
            for r in range(len(RB))
        ]
        for r, (R0, nr) in enumerate(RB):
            nc.gpsimd.partition_broadcast(
                ab0[r][:, : nr * W], scr0[0:1, R0 * W : (R0 + nr) * W], channels=128
            )
        nc.gpsimd.partition_all_reduce(
            red8[:, W:], dots8[:, W:], channels=8, reduce_op=RO.add
        )
        nc.vector.reciprocal(red8[:, W:], red8[:, W:])
        rsm = [
            sb.tile([128, 512], f32, tag=f"rsm{r}", name=f"rsm{r}")
            for r in range(len(RB))
        ]
        for r, (R0, nr) in enumerate(RB):
            nc.gpsimd.partition_broadcast(
                rsm[r][:, : nr * W], red8[0:1, R0 * W : (R0 + nr) * W], channels=128
            )

        nc.vector.memset(pooled_h[:, :, 0:1], 0.0)
        nc.vector.memset(pooled_h[:, :, W + 1 : W + 2], 0.0)
        nc.vector.memset(pooled_h[:, 0:1, :], 0.0)

        def vapply(t, vps, abt):
            # pooled += exp_t (broadcast over channels) * v_t, straight from
            # the conv PSUM banks; the last frame normalizes into the fp16
            # copy the out-conv consumes. exp_t reaches all 128 partitions
            # via a row-move DMA to partition 0 + gpsimd partition_broadcast.
            if abt is None:
                scr = scrpool.tile([1, NPIX], f32, tag="scr", name=f"scr{t}")
                nc.sync.dma_start(out=scr[:, W:], in_=dots8[t : t + 1, W:])
                abt = []
                for r, (R0, nr) in enumerate(RB):
                    ab = abpool.tile([128, 512], f32, tag="ab", name=f"ab{t}_{r}")
                    nc.gpsimd.partition_broadcast(
                        ab[:, : nr * W], scr[0:1, R0 * W : (R0 + nr) * W], channels=128
                    )
                    abt.append(ab)
            for r, (R0, nr) in enumerate(RB):
                rows = slice(R0, R0 + nr)
                cols = slice(R0 * W, (R0 + nr) * W)
                ab = abt[r]
                if t == 0:
                    nc.vector.tensor_mul(
                        pooled[:, rows, 1 : W + 1],
                        vps[r][:, : nr * W].rearrange("p (r c) -> p r c", c=W),
                        ab[:, : nr * W].rearrange("p (r c) -> p r c", c=W),
                    )
                    continue
                u = qkpool.tile([128, NPIX], f32, tag="qk", name=f"u{t}_{r}")
                nc.vector.tensor_mul(u[:, cols], vps[r][:, : nr * W], ab[:, : nr * W])
                if t < T - 1:
                    nc.vector.tensor_add(
                        pooled[:, rows, 1 : W + 1],
                        pooled[:, rows, 1 : W + 1],
                        u[:, cols].rearrange("p (r c) -> p r c", c=W),
                    )
                else:
                    u3 = u[:, cols].rearrange("p (r c) -> p r c", c=W)
                    nc.vector.tensor_add(u3, pooled[:, rows, 1 : W + 1], u3)
                    nc.vector.tensor_mul(
                        pooled_h[:, rows, 1 : W + 1],
                        u3,
                        rsm[r][:, : nr * W].rearrange("p (r c) -> p r c", c=W),
                    )

        # ---- phase 3: v convs with attn-weighted accumulation ----
        vapply(0, vps0, ab0)
        for t in range(1, T):
            vps = vconv(t, load_x(t))
            vapply(t, vps, None)

        # ---- phase 4: out = conv(pooled_h, w_out) + b ----
        if True:
            for R0o, nr in OUT_RB:
                for g in range(2):
                    op = psc.tile([128, 512], f32, tag="cv", name=f"op{R0o}_{g}")
                    for j in range(9):
                        ky, kx = divmod(j, 3)
                        nc.tensor.matmul(
                            op[:, : nr * W],
                            wo_sb[:, j, g * 128 : (g + 1) * 128],
                            pooled_h[:, R0o + ky : R0o + ky + nr, kx : kx + W],
                            start=(j == 0),
                            stop=(j == 8),
                        )
                    nc.scalar.add(
                        out_sb[:, g, R0o * W : (R0o + nr) * W],
                        op[:, : nr * W],
                        bo_sb[:, g : g + 1],
                    )
                    if R0o == OUT_RB[-1][0] and g == 1:
                        # final block leaves via the prepared kv_writeback
                        continue
                    sl = slice(R0o * W, (R0o + nr) * W)
                    # the (24,6) DMAs issue from the ACT hwdge queue so
                    # SP's issue path is clear for the (30,2) g0 DMA
                    # (SP has the shorter DGE delay)
                    eng = nc.scalar if R0o == 24 else nc.sync
                    eng.dma_start(out=out_d[g, :, sl], in_=out_sb[:, g, sl])
            # guard op: reads the final block's range so Tile orders the
            # trigger (same engine, program order) after the bias-add
            nc.gpsimd.partition_all_reduce(
                kvscr, out_sb[:, 1, 30 * W : 32 * W], channels=128,
                reduce_op=RO.add,
            )
            nc.gpsimd.trigger_dma(count=1)

    nc.compile()
    return nc


def _get_nc():
    if "nc" not in _cache:
        _cache["nc"] = _build_nc()
    return _cache["nc"]


def _shared_inputs(w_k, w_q, w_v, w_out, b_out, flip):
    def prep(w):  # optionally flip the kh tap axis (lower-half cores)
        w = np.asarray(w, np.float32)
        return w[:, :, ::-1, :] if flip else w

    def conv_lhst(w):  # (co=128, ci=256, 3, 3) -> (ci128, g, j, co)
        return np.ascontiguousarray(
            prep(w)
            .reshape(128, 2, 128, 3, 3)
            .transpose(2, 1, 3, 4, 0)
            .reshape(128, 2, 9, 128)
        ).astype(np.float16)

    wo = np.ascontiguousarray(  # (co=256, dh=128, 3, 3) -> (dh, j, co)
        prep(w_out).transpose(1, 2, 3, 0).reshape(128, 9, 256)
    ).astype(np.float16)
    bo = np.ascontiguousarray(np.asarray(b_out, np.float32).reshape(2, 128).T)
    return {
        "wq": conv_lhst(w_q),
        "wk": conv_lhst(w_k),
        "wv": conv_lhst(w_v),
        "wo": wo,
        "bo": bo,
    }


def core_inputs(c, x, shared):
    b, half = divmod(c, 2)
    # lower-half cores see the image vertically flipped, so the dead
    # out-of-image row is compute row 0 on every core
    xb = np.asarray(x, np.float32)[b]
    if half == 1:
        xb = xb[:, :, ::-1, :]
    xp = np.zeros((T, C, XR, WP), np.float16)
    lo, hi = -2, XR - 2
    slo = 0
    shi = min(hi, H)
    xp[:, :, slo - lo : slo - lo + (shi - slo), 1 : W + 1] = xb[
        :, :, slo:shi, :
    ].astype(np.float16)
    xs = xp.reshape(T, 2, 128, XR * WP)
    return {"xs": xs, **shared[half]}


def kernel(x, w_k, w_q, w_v, w_out, b_out):
    global LAST_RESULT
    from concourse.bass_utils import run_bass_kernel_spmd

    nc = _get_nc()
    shared = [
        _shared_inputs(w_k, w_q, w_v, w_out, b_out, flip) for flip in (False, True)
    ]
    in_maps = [core_inputs(c, x, shared) for c in range(NCORES)]
    res = run_bass_kernel_spmd(
        nc, in_maps, core_ids=list(range(NCORES)), **RUN_KWARGS
    )
    LAST_RESULT = res

    out = np.empty((B, C, H, W), np.float32)
    for c in range(NCORES):
        b, half = divmod(c, 2)
        o = res.results[c]["out"].reshape(C, 32, W)
        if half == 1:
            o = o[:, ::-1, :]
        out[b, :, half * 32 : half * 32 + 32, :] = o
    return np.broadcast_to(out[:, None], (B, T, C, H, W))
